# revision 25
# baseline (speedup 1.0000x reference)
"""Trainium2 Bass kernel for DecoderCrossAttention (B=8, S=2048, T=1024, E=1024, C=768, H=16, D=64).

Data-parallel over batch: 8 NeuronCores, one batch element each, no collectives.

v2 design (vs v1 baseline):
  - host passes x/enc/weights pre-converted to bf16 (no on-device converts, half the DMA)
  - host precomputes bo_eff = bv @ Wo + bo
  - attn@V flipped to out[s,d] = sum_t eT[t,s] v[t,d] (M=128, half the PE columns) with a
    x16 ones-column appended to v -> softmax row sums fall out of the same matmuls
    (kills the old M=32 ones-matmul pass entirely); 16 == H so the reciprocal is
    directly the avg-attn weight r_h[s]/H
  - recip columns -> rows via tiny PE transposes; one grouped wb broadcast per 4 heads
  - attnV transposed back to [e',s] on PE, normalized during evacuation via one STT
    pass (x16 compensation folded into the STT scalar)
  - avg-attn accumulation split DVE (12 heads) / GpSimd-Pool (4 heads)
  - outputs written bf16; host converts to f32
"""

import sys

sys.path.insert(0, "/opt/trn_rl_repo")

from contextlib import ExitStack

import numpy as np

import concourse.bass as bass
import concourse.mybir as mybir
import concourse.tile as tile
from concourse import bacc
from concourse.bass_utils import run_bass_kernel_spmd
from concourse.masks import make_identity

F32 = mybir.dt.float32
BF16 = mybir.dt.bfloat16
AF = mybir.ActivationFunctionType
OP = mybir.AluOpType

N_CORES = 8
S, T, E, C = 2048, 1024, 1024, 768
H, D = 16, 64
P = 128
SC = 256  # S-chunk size
NCH = S // SC
KE = E // P  # 8
KC = C // P  # 6
TT = T // P  # 8
SCALE = 0.125
HVAL = 16.0  # ones-column value == H: row sums arrive pre-scaled by H
POOL_HEADS = (0, 3, 6, 9, 12)  # avg-attn heads accumulated on GpSimd instead of DVE

_PROGRAM = None


def _bcast_dma(nc, out_t, src_row):
    """Broadcast a [1, F...] SBUF row to out_t [rows, F...] via zero-step DMA."""
    src_b = bass.AP(
        tensor=src_row.tensor,
        offset=src_row.offset,
        ap=[list(src_row.ap[0]), [0, out_t.shape[0]]]
        + [list(d) for d in src_row.ap[1:]],
    )
    nc.sync.dma_start(out=out_t, in_=src_b)


def build_program():
    nc = bacc.Bacc("TRN2", target_bir_lowering=False, debug=False, num_devices=N_CORES)

    xb = nc.dram_tensor("x", [S, E], BF16, kind="ExternalInput").ap()
    encb = nc.dram_tensor("enc", [T, C], BF16, kind="ExternalInput").ap()
    Wq = nc.dram_tensor("Wq", [E, E], BF16, kind="ExternalInput").ap()
    Wk = nc.dram_tensor("Wk", [C, E], BF16, kind="ExternalInput").ap()
    Wv = nc.dram_tensor("Wv", [C, E], BF16, kind="ExternalInput").ap()
    Wo = nc.dram_tensor("Wo", [E, E], BF16, kind="ExternalInput").ap()
    bq = nc.dram_tensor("bq", [E], F32, kind="ExternalInput").ap()
    bk = nc.dram_tensor("bk", [E], F32, kind="ExternalInput").ap()
    boe = nc.dram_tensor("boe", [E], BF16, kind="ExternalInput").ap()
    out = nc.dram_tensor("out", [S, E], BF16, kind="ExternalOutput").ap()
    avg = nc.dram_tensor("avg", [S, T], BF16, kind="ExternalOutput").ap()

    with tile.TileContext(nc) as tc:
        _build(tc, xb, encb, Wq, Wk, Wv, Wo, bq, bk, boe, out, avg)
    nc.compile()
    return nc


def _build(tc, xb, encb, Wq, Wk, Wv, Wo, bq, bk, boe, out, avg):
    nc = tc.nc
    with ExitStack() as stack:
        consts = stack.enter_context(tc.tile_pool(name="consts", bufs=1))
        resident = stack.enter_context(tc.tile_pool(name="resident", bufs=1))

        ident_b = consts.tile([P, P], BF16)
        make_identity(nc, ident_b)
        ones_row = consts.tile([1, P], BF16)
        nc.vector.memset(ones_row, 1.0)
        bq_sb = consts.tile([P, KE], F32)
        nc.sync.dma_start(out=bq_sb, in_=bq.rearrange("(m p) -> p m", p=P))
        bk_sb = consts.tile([P, KE], F32)
        nc.sync.dma_start(out=bk_sb, in_=bk.rearrange("(m p) -> p m", p=P))
        boe_row = consts.tile([1, E], BF16)
        nc.sync.dma_start(out=boe_row, in_=boe[None, :])

        Wq_bf = resident.tile([P, KE, E], BF16)
        Wo_bf = resident.tile([P, KE, E], BF16)
        kT_bf = resident.tile([P, KE, T], BF16)
        v0x = resident.tile([P, TT, H, 65], BF16)  # [t, t8, head, d | sum-col]
        nc.vector.memset(v0x[:, :, :, 64:65], HVAL)

        # ---------------- phase A: K/V projections ----------------
        with ExitStack() as ph:
            early = ph.enter_context(tc.tile_pool(name="early", bufs=1))
            ld_pool = ph.enter_context(tc.tile_pool(name="ld_pool", bufs=3))
            ph_ps = ph.enter_context(tc.tile_pool(name="ph_ps", bufs=2, space="PSUM"))
            ph_tr = ph.enter_context(tc.tile_pool(name="ph_tr", bufs=2, space="PSUM"))

            # DMA order matters: enc tiles feed the first PE ops; big weight
            # loads go after the tensors needed soonest.
            enc_tiles = []
            for t8 in range(TT):
                enc_t = ld_pool.tile([P, C], BF16, tag="ld", name="enc_t", bufs=8)
                nc.sync.dma_start(out=enc_t, in_=encb[t8 * P : (t8 + 1) * P, :])
                enc_tiles.append(enc_t)
            Wk_bf = early.tile([P, KC, E], BF16)
            nc.sync.dma_start(out=Wk_bf, in_=Wk.rearrange("(k p) e -> p k e", p=P))
            Wv_bf = early.tile([P, KC, E], BF16)
            nc.sync.dma_start(out=Wv_bf, in_=Wv.rearrange("(k p) e -> p k e", p=P))
            nc.sync.dma_start(out=Wq_bf, in_=Wq.rearrange("(k p) e -> p k e", p=P))
            nc.sync.dma_start(out=Wo_bf, in_=Wo.rearrange("(k p) e -> p k e", p=P))
            encT_bf = early.tile([P, KC, T], BF16)

            # enc -> PE-transpose -> encT
            for t8 in range(TT):
                enc_t = enc_tiles[t8]
                ps_a = ph_tr.tile([P, C], BF16, tag="phb", name="ps_tr")
                for c6 in range(KC):
                    nc.tensor.transpose(
                        ps_a[:, c6 * P : (c6 + 1) * P],
                        enc_t[:, c6 * P : (c6 + 1) * P],
                        ident_b,
                    )
                nc.vector.tensor_copy(
                    encT_bf[:, :, t8 * P : (t8 + 1) * P],
                    ps_a.rearrange("p (k t) -> p k t", k=KC),
                )

            # kT[e',t] = sum_c Wk[c,e'] encT[c,t], +bk
            for m8 in range(KE):
                ps_a = ph_ps.tile([P, T], F32, tag="ph", name="ps_k")
                for c6 in range(KC):
                    for n2 in range(2):
                        nc.tensor.matmul(
                            ps_a[:, n2 * 512 : (n2 + 1) * 512],
                            Wk_bf[:, c6, m8 * P : (m8 + 1) * P],
                            encT_bf[:, c6, n2 * 512 : (n2 + 1) * 512],
                            start=(c6 == 0),
                            stop=(c6 == KC - 1),
                        )
                nc.scalar.activation(
                    kT_bf[:, m8, :], ps_a, AF.Identity, bias=bk_sb[:, m8 : m8 + 1]
                )

            # v0[t,e'] = sum_c encT[c,t] Wv[c,e']  (bv folded into boe on host)
            for t8 in range(TT):
                ps_a = ph_ps.tile([P, T], F32, tag="ph", name="ps_v")
                for c6 in range(KC):
                    for n2 in range(2):
                        nc.tensor.matmul(
                            ps_a[:, n2 * 512 : (n2 + 1) * 512],
                            encT_bf[:, c6, t8 * P : (t8 + 1) * P],
                            Wv_bf[:, c6, n2 * 512 : (n2 + 1) * 512],
                            start=(c6 == 0),
                            stop=(c6 == KC - 1),
                        )
                nc.vector.tensor_copy(
                    v0x[:, t8, :, 0:64],
                    ps_a.rearrange("p (h d) -> p h d", h=H),
                )

        # ---------------- phase B: S-chunk loop ----------------
        with ExitStack() as mn:
            xload = mn.enter_context(tc.tile_pool(name="xload", bufs=2))
            xt_pool = mn.enter_context(tc.tile_pool(name="xt_pool", bufs=2))
            qt_pool = mn.enter_context(tc.tile_pool(name="qt_pool", bufs=2))
            et_pool = mn.enter_context(tc.tile_pool(name="et_pool", bufs=6))
            wb_pool = mn.enter_context(tc.tile_pool(name="wb_pool", bufs=3))
            rq_pool = mn.enter_context(tc.tile_pool(name="rq_pool", bufs=3))
            att_pool = mn.enter_context(tc.tile_pool(name="att_pool", bufs=2))
            acc_pool = mn.enter_context(tc.tile_pool(name="acc_pool", bufs=2))
            ot_pool = mn.enter_context(tc.tile_pool(name="ot_pool", bufs=2))
            sb_pool = mn.enter_context(tc.tile_pool(name="sb_pool", bufs=2))

            ps_scores = mn.enter_context(
                tc.tile_pool(name="ps_scores", bufs=2, space="PSUM")
            )
            ps_avp = mn.enter_context(tc.tile_pool(name="ps_avp", bufs=1, space="PSUM"))
            ps_misc = mn.enter_context(tc.tile_pool(name="ps_misc", bufs=2, space="PSUM"))

            def emit_outproj(st):
                s0p, outT_p = st["s0"], st["outT"]
                # final projection: out[s,e] = outT^T @ Wo + boe
                for m2 in range(2):
                    out_sb = sb_pool.tile([P, E], BF16, tag="out_sb", name="out_sb")
                    for n2 in range(2):
                        ps_m = ps_misc.tile([P, 512], F32, tag="misc", name="ps_o")
                        for k8 in range(KE):
                            nc.tensor.matmul(
                                ps_m,
                                outT_p[:, k8, m2 * P : (m2 + 1) * P],
                                Wo_bf[:, k8, n2 * 512 : (n2 + 1) * 512],
                                start=(k8 == 0),
                                stop=False,
                            )
                        nc.tensor.matmul(
                            ps_m,
                            ones_row,
                            boe_row[:, n2 * 512 : (n2 + 1) * 512],
                            start=False,
                            stop=True,
                        )
                        if n2 == 0:
                            nc.scalar.copy(out_sb[:, n2 * 512 : (n2 + 1) * 512], ps_m)
                        else:
                            nc.vector.tensor_copy(
                                out_sb[:, n2 * 512 : (n2 + 1) * 512], ps_m
                            )
                    nc.sync.dma_start(
                        out=out[s0p + m2 * P : s0p + (m2 + 1) * P, :], in_=out_sb
                    )

            def emit_avgout(st):
                s0p, acc_v_p, acc_p_p = st["s0"], st["acc_v"], st["acc_p"]
                # avg_attn: merge acc halves, PE-transpose back to [s, t]
                acc_m = acc_pool.tile([P, TT, SC], BF16, tag="accm", name="acc_m", bufs=1)
                nc.vector.tensor_tensor(acc_m, acc_v_p, acc_p_p, OP.add)
                for s2 in range(2):
                    ps_m = ps_misc.tile([P, T], BF16, tag="misc", name="ps_av_t")
                    for t8 in range(TT):
                        nc.tensor.transpose(
                            ps_m[:, t8 * P : (t8 + 1) * P],
                            acc_m[:, t8, s2 * P : (s2 + 1) * P],
                            ident_b,
                        )
                    avg_sb = sb_pool.tile([P, T], BF16, tag="avg_sb", name="avg_sb")
                    nc.scalar.copy(avg_sb, ps_m)
                    nc.sync.dma_start(
                        out=avg[s0p + s2 * P : s0p + (s2 + 1) * P, :], in_=avg_sb
                    )

            def emit_scores_half(st, pr, half):
                if half == 0:
                    st["eT"][pr] = et_pool.tile(
                        [P, 2, TT, SC], BF16, tag="eT", name="eT_pair"
                    )
                ps_sc = {
                    0: ps_scores.tile([P, 4, SC], F32, tag="scores", name="ps_sc_a"),
                    1: ps_scores.tile([P, 4, SC], F32, tag="scores", name="ps_sc_b"),
                }
                for t4 in range(4):
                    t8 = half * 4 + t4
                    for hh, tp in ((0, 0), (1, 64)):
                        nc.tensor.matmul(
                            ps_sc[hh][:, t4, :],
                            kT_bf[tp : tp + D, pr, t8 * P : (t8 + 1) * P],
                            st["qT_c"][tp : tp + D, pr, :],
                            start=True,
                            stop=True,
                            tile_position=(tp, 0),
                        )
                for hh in range(2):
                    nc.scalar.activation(
                        st["eT"][pr][:, hh, half * 4 : half * 4 + 4, :],
                        ps_sc[hh],
                        AF.Exp,
                        scale=SCALE,
                    )

            def emit_attnv_units(st, pr, units):
                g, pl = divmod(pr, 2)
                for u in units:
                    hh, s2 = divmod(u, 2)
                    if u == 0 and pl == 0:
                        st["ps_av"][g] = ps_avp.tile(
                            [P, 2, 4, P], F32, tag="po", name="ps_av"
                        )
                    h = 2 * pr + hh
                    j = 2 * pl + hh
                    for t8 in range(TT):
                        nc.tensor.matmul(
                            st["ps_av"][g][:, s2, j, 0:65],
                            st["eT"][pr][:, hh, t8, s2 * P : (s2 + 1) * P],
                            v0x[:, t8, h, :],
                            start=(t8 == 0),
                            stop=(t8 == TT - 1),
                        )

            def emit_groupnorm(st, g):
                # reciprocal of the x16 row sums -> r_h[s]/16 == r_h[s]/H
                pg = st["ps_av"].pop(g)
                rqr = rq_pool.tile([P, 2, 4], F32, tag="rqr", name="rqr")
                nc.vector.reciprocal_approx_fast(
                    out=rqr,
                    in_=pg[:, :, :, 64:65].rearrange("p a b c -> p a (b c)"),
                )
                rqb = rq_pool.tile([P, 2, 4], BF16, tag="rqb", name="rqb")
                nc.vector.tensor_copy(rqb, rqr)
                # column -> row transposes, then per-head broadcasts
                ps_r = ps_misc.tile([4, 2, P], BF16, tag="misc", name="ps_rq")
                for s2 in range(2):
                    nc.tensor.transpose(ps_r[:, s2, :], rqb[:, s2, :], ident_b)
                rq_rows = rq_pool.tile([4, 2, P], BF16, tag="rqrow", name="rq_rows")
                nc.vector.tensor_copy(rq_rows, ps_r)
                wb_g = wb_pool.tile([P, 4, 2, P], BF16, tag="wb", name="wb_g")
                st["wb"][g] = wb_g
                for jj in range(4):
                    _bcast_dma(nc, wb_g[:, jj, :, :], rq_rows[jj : jj + 1, :, :])

                # attnV evacuation (raw), then transpose + normalize into outT
                nc.scalar.copy(
                    st["attnV_sb"][:, :, 4 * g : 4 * g + 4, :], pg[:, :, :, 0:64]
                )
                ps_t = ps_misc.tile([P, 2, 2, P], BF16, tag="misc", name="ps_ot")
                for pl in range(2):
                    pr = 2 * g + pl
                    for s2 in range(2):
                        nc.tensor.transpose(
                            ps_t[:, pl, s2, :],
                            st["attnV_sb"][:, s2, 2 * pr : 2 * pr + 2, :].rearrange(
                                "p h d -> p (h d)"
                            ),
                            ident_b,
                        )
                for pl in range(2):
                    pr = 2 * g + pl
                    for hh, lo in ((0, 0), (1, 64)):
                        nc.vector.scalar_tensor_tensor(
                            st["outT"][lo : lo + 64, pr, :].rearrange(
                                "p (a s) -> p a s", a=2
                            ),
                            ps_t[lo : lo + 64, pl, :, :],
                            HVAL,
                            wb_g[lo : lo + 64, 2 * pl + hh, :, :],
                            OP.mult,
                            OP.mult,
                        )

            def emit_groupavg(st, g):
                # avg-attn accumulation for the 4 heads of this group
                wb_g = st["wb"].pop(g)
                for jj in range(4):
                    h = 4 * g + jj
                    pr2, hh = divmod(h, 2)
                    eT_h = st["eT"][pr2][:, hh]
                    wb_b = (
                        wb_g[:, jj, :, :]
                        .rearrange("p a s -> p (a s)")[:, None, :]
                        .to_broadcast([P, TT, SC])
                    )
                    if h in POOL_HEADS:
                        eng, acc, first = nc.gpsimd, st["acc_p"], st["first_p"]
                    else:
                        eng, acc, first = nc.vector, st["acc_v"], st["first_v"]
                    if first[0]:
                        eng.tensor_tensor(acc, eT_h, wb_b, OP.mult)
                        first[0] = False
                    else:
                        tag = "tmp_p" if h in POOL_HEADS else "tmp_v"
                        tmp = acc_pool.tile(
                            [P, TT, SC], BF16, tag=tag, name="tmp_h", bufs=2
                        )
                        eng.tensor_tensor(tmp, eT_h, wb_b, OP.mult)
                        eng.tensor_tensor(acc, acc, tmp, OP.add)

            prev_st = None
            for c in range(NCH):
                s0 = c * SC
                x_sb = xload.tile([P, 2, E], BF16, tag="x_sb", name="x_sb")
                nc.sync.dma_start(
                    out=x_sb, in_=xb[s0 : s0 + SC, :].rearrange("(a p) e -> p a e", p=P)
                )
                xT_c = xt_pool.tile([P, KE, SC], BF16)
                for eh in range(2):  # halves of the e8 range
                    ps_x = ps_misc.tile([P, 4, 2, P], BF16, tag="misc", name="ps_xt")
                    for e4 in range(4):
                        e8 = eh * 4 + e4
                        for s2 in range(2):
                            nc.tensor.transpose(
                                ps_x[:, e4, s2, :],
                                x_sb[:, s2, e8 * P : (e8 + 1) * P],
                                ident_b,
                            )
                    nc.vector.tensor_copy(
                        xT_c[:, eh * 4 : (eh + 1) * 4, :].rearrange(
                            "p k (a s) -> p k a s", a=2
                        ),
                        ps_x,
                    )

                qT_c = qt_pool.tile([P, KE, SC], BF16)
                for mh in range(4):  # pairs of m8
                    ps_q = ps_misc.tile([P, 2, SC], F32, tag="misc", name="ps_q")
                    for i in range(2):
                        m8 = mh * 2 + i
                        for k8 in range(KE):
                            nc.tensor.matmul(
                                ps_q[:, i, :],
                                Wq_bf[:, k8, m8 * P : (m8 + 1) * P],
                                xT_c[:, k8, :],
                                start=(k8 == 0),
                                stop=(k8 == KE - 1),
                            )
                    for i in range(2):
                        m8 = mh * 2 + i
                        nc.vector.tensor_scalar(
                            qT_c[:, m8, :],
                            ps_q[:, i, :],
                            bq_sb[:, m8 : m8 + 1],
                            None,
                            OP.add,
                        )

                # deferred work of the previous chunk, interleaved for latency:
                # norms (small serial chains) first, projection next, bulk avg last
                if prev_st is not None:
                    emit_groupnorm(prev_st, 2)
                    emit_groupnorm(prev_st, 3)
                    emit_outproj(prev_st)
                    emit_groupavg(prev_st, 2)
                    emit_groupavg(prev_st, 3)

                st = {
                    "s0": s0,
                    "qT_c": qT_c,
                    "acc_v": acc_pool.tile([P, TT, SC], BF16, tag="accv", name="acc_v"),
                    "acc_p": acc_pool.tile([P, TT, SC], BF16, tag="accp", name="acc_p"),
                    "outT": ot_pool.tile([P, KE, SC], BF16, name="outT"),
                    "attnV_sb": att_pool.tile([P, 2, H, D], BF16, name="attnV_sb"),
                    "eT": {},
                    "ps_av": {},
                    "wb": {},
                    "first_v": [True],
                    "first_p": [True],
                }

                for pr in range(H // 2):
                    emit_scores_half(st, pr, 0)
                    if pr >= 1:
                        emit_attnv_units(st, pr - 1, (0, 1))
                    emit_scores_half(st, pr, 1)
                    if pr >= 1:
                        emit_attnv_units(st, pr - 1, (2, 3))
                    if pr == 1 and prev_st is not None:
                        emit_avgout(prev_st)
                    if pr in (3, 5):
                        g = (pr - 3) // 2
                        emit_groupnorm(st, g)
                        emit_groupavg(st, g)
                emit_attnv_units(st, H // 2 - 1, (0, 1, 2, 3))
                prev_st = st

            emit_groupnorm(prev_st, 2)
            emit_groupnorm(prev_st, 3)
            emit_outproj(prev_st)
            emit_groupavg(prev_st, 2)
            emit_groupavg(prev_st, 3)
            emit_avgout(prev_st)


def get_program():
    global _PROGRAM
    if _PROGRAM is None:
        _PROGRAM = build_program()
    return _PROGRAM


def prep_inputs(inputs):
    """Host-side prep: bf16 conversion + bo_eff fold. Returns per-core input maps."""
    import ml_dtypes

    bf16 = ml_dtypes.bfloat16
    f32 = np.float32
    Wo = np.asarray(inputs["Wo"], dtype=f32)
    boe = np.asarray(inputs["bv"], dtype=f32) @ Wo + np.asarray(inputs["bo"], dtype=f32)
    common = {
        "Wq": np.ascontiguousarray(np.asarray(inputs["Wq"], dtype=f32).astype(bf16)),
        "Wk": np.ascontiguousarray(np.asarray(inputs["Wk"], dtype=f32).astype(bf16)),
        "Wv": np.ascontiguousarray(np.asarray(inputs["Wv"], dtype=f32).astype(bf16)),
        "Wo": np.ascontiguousarray(Wo.astype(bf16)),
        "bq": np.ascontiguousarray(np.asarray(inputs["bq"], dtype=f32)),
        "bk": np.ascontiguousarray(np.asarray(inputs["bk"], dtype=f32)),
        "boe": np.ascontiguousarray(boe.astype(bf16)),
    }
    x = np.asarray(inputs["x"], dtype=f32).astype(bf16)
    enc = np.asarray(inputs["encoder_output"], dtype=f32).astype(bf16)
    return [
        dict(common, x=np.ascontiguousarray(x[b]), enc=np.ascontiguousarray(enc[b]))
        for b in range(N_CORES)
    ]


def kernel(**inputs):
    nc = get_program()
    in_maps = prep_inputs(inputs)
    res = run_bass_kernel_spmd(nc, in_maps, list(range(N_CORES)))
    out = np.stack(
        [np.asarray(res.results[b]["out"]).astype(np.float32) for b in range(N_CORES)]
    )
    avg = np.stack(
        [np.asarray(res.results[b]["avg"]).astype(np.float32) for b in range(N_CORES)]
    )
    return out, avg


# revision 26
# speedup vs baseline: 1.1786x; 1.1786x over previous
"""Trainium2 Bass kernel for DecoderCrossAttention (B=8, S=2048, T=1024, E=1024, C=768, H=16, D=64).

Data-parallel over batch: 8 NeuronCores, one batch element each, no collectives.

v2 design (vs v1 baseline):
  - host passes x/enc/weights pre-converted to bf16 (no on-device converts, half the DMA)
  - host precomputes bo_eff = bv @ Wo + bo
  - attn@V flipped to out[s,d] = sum_t eT[t,s] v[t,d] (M=128, half the PE columns) with a
    x16 ones-column appended to v -> softmax row sums fall out of the same matmuls
    (kills the old M=32 ones-matmul pass entirely); 16 == H so the reciprocal is
    directly the avg-attn weight r_h[s]/H
  - recip columns -> rows via tiny PE transposes; one grouped wb broadcast per 4 heads
  - attnV transposed back to [e',s] on PE, normalized during evacuation via one STT
    pass (x16 compensation folded into the STT scalar)
  - avg-attn accumulation split DVE (12 heads) / GpSimd-Pool (4 heads)
  - outputs written bf16; host converts to f32
"""

import sys

sys.path.insert(0, "/opt/trn_rl_repo")

from contextlib import ExitStack

import numpy as np

import concourse.bass as bass
import concourse.mybir as mybir
import concourse.tile as tile
from concourse import bacc
from concourse.bass_utils import run_bass_kernel_spmd
from concourse.masks import make_identity

F32 = mybir.dt.float32
BF16 = mybir.dt.bfloat16
AF = mybir.ActivationFunctionType
OP = mybir.AluOpType

N_CORES = 8
S, T, E, C = 2048, 1024, 1024, 768
H, D = 16, 64
P = 128
SC = 256  # S-chunk size
NCH = S // SC
KE = E // P  # 8
KC = C // P  # 6
TT = T // P  # 8
SCALE = 0.125
HVAL = 16.0  # ones-column value == H: row sums arrive pre-scaled by H
POOL_HEADS = (0, 3, 6, 9, 12)  # avg-attn heads accumulated on GpSimd instead of DVE

_PROGRAM = None


def _bcast_dma(nc, out_t, src_row):
    """Broadcast a [1, F...] SBUF row to out_t [rows, F...] via zero-step DMA."""
    src_b = bass.AP(
        tensor=src_row.tensor,
        offset=src_row.offset,
        ap=[list(src_row.ap[0]), [0, out_t.shape[0]]]
        + [list(d) for d in src_row.ap[1:]],
    )
    nc.sync.dma_start(out=out_t, in_=src_b)


def build_program(loop_iters=0):
    nc = bacc.Bacc("TRN2", target_bir_lowering=False, debug=False, num_devices=N_CORES)

    xb = nc.dram_tensor("x", [S, E], BF16, kind="ExternalInput").ap()
    encb = nc.dram_tensor("enc", [T, C], BF16, kind="ExternalInput").ap()
    Wq = nc.dram_tensor("Wq", [E, E], BF16, kind="ExternalInput").ap()
    Wk = nc.dram_tensor("Wk", [C, E], BF16, kind="ExternalInput").ap()
    Wv = nc.dram_tensor("Wv", [C, E], BF16, kind="ExternalInput").ap()
    Wo = nc.dram_tensor("Wo", [E, E], BF16, kind="ExternalInput").ap()
    bq = nc.dram_tensor("bq", [E], F32, kind="ExternalInput").ap()
    bk = nc.dram_tensor("bk", [E], F32, kind="ExternalInput").ap()
    boe = nc.dram_tensor("boe", [E], BF16, kind="ExternalInput").ap()
    out = nc.dram_tensor("out", [S, E], BF16, kind="ExternalOutput").ap()
    avg = nc.dram_tensor("avg", [S, T], BF16, kind="ExternalOutput").ap()

    with tile.TileContext(nc) as tc:
        if loop_iters:
            with tc.For_i(0, loop_iters, 1):
                _build(tc, xb, encb, Wq, Wk, Wv, Wo, bq, bk, boe, out, avg)
        else:
            _build(tc, xb, encb, Wq, Wk, Wv, Wo, bq, bk, boe, out, avg)
    nc.compile()
    return nc


def _build(tc, xb, encb, Wq, Wk, Wv, Wo, bq, bk, boe, out, avg):
    nc = tc.nc
    with ExitStack() as stack:
        consts = stack.enter_context(tc.tile_pool(name="consts", bufs=1))
        resident = stack.enter_context(tc.tile_pool(name="resident", bufs=1))

        ident_b = consts.tile([P, P], BF16)
        make_identity(nc, ident_b)
        ones_row = consts.tile([1, P], BF16)
        nc.vector.memset(ones_row, 1.0)
        bq_sb = consts.tile([P, KE], F32)
        nc.sync.dma_start(out=bq_sb, in_=bq.rearrange("(m p) -> p m", p=P))
        bk_sb = consts.tile([P, KE], F32)
        nc.sync.dma_start(out=bk_sb, in_=bk.rearrange("(m p) -> p m", p=P))
        boe_row = consts.tile([1, E], BF16)
        nc.sync.dma_start(out=boe_row, in_=boe[None, :])

        Wq_bf = resident.tile([P, KE, E], BF16)
        Wo_bf = resident.tile([P, KE, E], BF16)
        kT_bf = resident.tile([P, KE, T], BF16)
        v0x = resident.tile([P, TT, H, 65], BF16)  # [t, t8, head, d | sum-col]
        nc.vector.memset(v0x[:, :, :, 64:65], HVAL)

        # ---------------- phase A: K/V projections ----------------
        with ExitStack() as ph:
            early = ph.enter_context(tc.tile_pool(name="early", bufs=1))
            ld_pool = ph.enter_context(tc.tile_pool(name="ld_pool", bufs=3))
            ph_ps = ph.enter_context(tc.tile_pool(name="ph_ps", bufs=2, space="PSUM"))
            ph_tr = ph.enter_context(tc.tile_pool(name="ph_tr", bufs=2, space="PSUM"))

            # DMA order matters: enc tiles feed the first PE ops; big weight
            # loads go after the tensors needed soonest.
            enc_tiles = []
            for t8 in range(TT):
                enc_t = ld_pool.tile([P, C], BF16, tag="ld", name="enc_t", bufs=8)
                nc.sync.dma_start(out=enc_t, in_=encb[t8 * P : (t8 + 1) * P, :])
                enc_tiles.append(enc_t)
            Wk_bf = early.tile([P, KC, E], BF16)
            nc.sync.dma_start(out=Wk_bf, in_=Wk.rearrange("(k p) e -> p k e", p=P))
            Wv_bf = early.tile([P, KC, E], BF16)
            nc.sync.dma_start(out=Wv_bf, in_=Wv.rearrange("(k p) e -> p k e", p=P))
            nc.sync.dma_start(out=Wq_bf, in_=Wq.rearrange("(k p) e -> p k e", p=P))
            nc.sync.dma_start(out=Wo_bf, in_=Wo.rearrange("(k p) e -> p k e", p=P))
            encT_bf = early.tile([P, KC, T], BF16)

            # enc -> PE-transpose -> encT
            for t8 in range(TT):
                enc_t = enc_tiles[t8]
                ps_a = ph_tr.tile([P, C], BF16, tag="phb", name="ps_tr")
                for c6 in range(KC):
                    nc.tensor.transpose(
                        ps_a[:, c6 * P : (c6 + 1) * P],
                        enc_t[:, c6 * P : (c6 + 1) * P],
                        ident_b,
                    )
                nc.vector.tensor_copy(
                    encT_bf[:, :, t8 * P : (t8 + 1) * P],
                    ps_a.rearrange("p (k t) -> p k t", k=KC),
                )

            # kT[e',t] = sum_c Wk[c,e'] encT[c,t], +bk
            for m8 in range(KE):
                ps_a = ph_ps.tile([P, T], F32, tag="ph", name="ps_k")
                for c6 in range(KC):
                    for n2 in range(2):
                        nc.tensor.matmul(
                            ps_a[:, n2 * 512 : (n2 + 1) * 512],
                            Wk_bf[:, c6, m8 * P : (m8 + 1) * P],
                            encT_bf[:, c6, n2 * 512 : (n2 + 1) * 512],
                            start=(c6 == 0),
                            stop=(c6 == KC - 1),
                        )
                nc.scalar.activation(
                    kT_bf[:, m8, :], ps_a, AF.Identity, bias=bk_sb[:, m8 : m8 + 1]
                )

            # v0[t,e'] = sum_c encT[c,t] Wv[c,e']  (bv folded into boe on host)
            for t8 in range(TT):
                ps_a = ph_ps.tile([P, T], F32, tag="ph", name="ps_v")
                for c6 in range(KC):
                    for n2 in range(2):
                        nc.tensor.matmul(
                            ps_a[:, n2 * 512 : (n2 + 1) * 512],
                            encT_bf[:, c6, t8 * P : (t8 + 1) * P],
                            Wv_bf[:, c6, n2 * 512 : (n2 + 1) * 512],
                            start=(c6 == 0),
                            stop=(c6 == KC - 1),
                        )
                nc.vector.tensor_copy(
                    v0x[:, t8, :, 0:64],
                    ps_a.rearrange("p (h d) -> p h d", h=H),
                )

        # ---------------- phase B: S-chunk loop ----------------
        with ExitStack() as mn:
            xload = mn.enter_context(tc.tile_pool(name="xload", bufs=2))
            xt_pool = mn.enter_context(tc.tile_pool(name="xt_pool", bufs=2))
            qt_pool = mn.enter_context(tc.tile_pool(name="qt_pool", bufs=2))
            et_pool = mn.enter_context(tc.tile_pool(name="et_pool", bufs=6))
            wb_pool = mn.enter_context(tc.tile_pool(name="wb_pool", bufs=3))
            rq_pool = mn.enter_context(tc.tile_pool(name="rq_pool", bufs=3))
            att_pool = mn.enter_context(tc.tile_pool(name="att_pool", bufs=2))
            acc_pool = mn.enter_context(tc.tile_pool(name="acc_pool", bufs=2))
            ot_pool = mn.enter_context(tc.tile_pool(name="ot_pool", bufs=2))
            sb_pool = mn.enter_context(tc.tile_pool(name="sb_pool", bufs=2))

            ps_scores = mn.enter_context(
                tc.tile_pool(name="ps_scores", bufs=2, space="PSUM")
            )
            ps_avp = mn.enter_context(tc.tile_pool(name="ps_avp", bufs=1, space="PSUM"))
            ps_misc = mn.enter_context(tc.tile_pool(name="ps_misc", bufs=2, space="PSUM"))

            def emit_outproj(st):
                s0p, outT_p = st["s0"], st["outT"]
                # final projection: out[s,e] = outT^T @ Wo + boe
                for m2 in range(2):
                    out_sb = sb_pool.tile([P, E], BF16, tag="out_sb", name="out_sb")
                    for n2 in range(2):
                        ps_m = ps_misc.tile([P, 512], F32, tag="misc", name="ps_o")
                        for k8 in range(KE):
                            nc.tensor.matmul(
                                ps_m,
                                outT_p[:, k8, m2 * P : (m2 + 1) * P],
                                Wo_bf[:, k8, n2 * 512 : (n2 + 1) * 512],
                                start=(k8 == 0),
                                stop=False,
                            )
                        nc.tensor.matmul(
                            ps_m,
                            ones_row,
                            boe_row[:, n2 * 512 : (n2 + 1) * 512],
                            start=False,
                            stop=True,
                        )
                        if n2 == 0:
                            nc.scalar.copy(out_sb[:, n2 * 512 : (n2 + 1) * 512], ps_m)
                        else:
                            nc.vector.tensor_copy(
                                out_sb[:, n2 * 512 : (n2 + 1) * 512], ps_m
                            )
                    nc.sync.dma_start(
                        out=out[s0p + m2 * P : s0p + (m2 + 1) * P, :], in_=out_sb
                    )

            def emit_avgout(st):
                s0p, acc_v_p, acc_p_p = st["s0"], st["acc_v"], st["acc_p"]
                # avg_attn: merge acc halves, PE-transpose back to [s, t]
                acc_m = acc_pool.tile([P, TT, SC], BF16, tag="accm", name="acc_m", bufs=1)
                nc.vector.tensor_tensor(acc_m, acc_v_p, acc_p_p, OP.add)
                for s2 in range(2):
                    ps_m = ps_misc.tile([P, T], BF16, tag="misc", name="ps_av_t")
                    for t8 in range(TT):
                        nc.tensor.transpose(
                            ps_m[:, t8 * P : (t8 + 1) * P],
                            acc_m[:, t8, s2 * P : (s2 + 1) * P],
                            ident_b,
                        )
                    avg_sb = sb_pool.tile([P, T], BF16, tag="avg_sb", name="avg_sb")
                    nc.scalar.copy(avg_sb, ps_m)
                    nc.sync.dma_start(
                        out=avg[s0p + s2 * P : s0p + (s2 + 1) * P, :], in_=avg_sb
                    )

            def emit_scores_half(st, pr, half):
                if half == 0:
                    st["eT"][pr] = et_pool.tile(
                        [P, 2, TT, SC], BF16, tag="eT", name="eT_pair"
                    )
                ps_sc = {
                    0: ps_scores.tile([P, 4, SC], F32, tag="scores", name="ps_sc_a"),
                    1: ps_scores.tile([P, 4, SC], F32, tag="scores", name="ps_sc_b"),
                }
                for t4 in range(4):
                    t8 = half * 4 + t4
                    for hh, tp in ((0, 0), (1, 64)):
                        nc.tensor.matmul(
                            ps_sc[hh][:, t4, :],
                            kT_bf[tp : tp + D, pr, t8 * P : (t8 + 1) * P],
                            st["qT_c"][tp : tp + D, pr, :],
                            start=True,
                            stop=True,
                            tile_position=(tp, 0),
                        )
                for hh in range(2):
                    nc.scalar.activation(
                        st["eT"][pr][:, hh, half * 4 : half * 4 + 4, :],
                        ps_sc[hh],
                        AF.Exp,
                        scale=SCALE,
                    )

            def emit_attnv_units(st, pr, units):
                g, pl = divmod(pr, 2)
                for u in units:
                    hh, s2 = divmod(u, 2)
                    if u == 0 and pl == 0:
                        st["ps_av"][g] = ps_avp.tile(
                            [P, 2, 4, P], F32, tag="po", name="ps_av"
                        )
                    h = 2 * pr + hh
                    j = 2 * pl + hh
                    for t8 in range(TT):
                        nc.tensor.matmul(
                            st["ps_av"][g][:, s2, j, 0:65],
                            st["eT"][pr][:, hh, t8, s2 * P : (s2 + 1) * P],
                            v0x[:, t8, h, :],
                            start=(t8 == 0),
                            stop=(t8 == TT - 1),
                        )

            def emit_groupnorm(st, g):
                # reciprocal of the x16 row sums -> r_h[s]/16 == r_h[s]/H
                pg = st["ps_av"].pop(g)
                rqr = rq_pool.tile([P, 2, 4], F32, tag="rqr", name="rqr")
                nc.vector.reciprocal_approx_fast(
                    out=rqr,
                    in_=pg[:, :, :, 64:65].rearrange("p a b c -> p a (b c)"),
                )
                rqb = rq_pool.tile([P, 2, 4], BF16, tag="rqb", name="rqb")
                nc.vector.tensor_copy(rqb, rqr)
                # column -> row transposes, then per-head broadcasts
                ps_r = ps_misc.tile([4, 2, P], BF16, tag="misc", name="ps_rq")
                for s2 in range(2):
                    nc.tensor.transpose(ps_r[:, s2, :], rqb[:, s2, :], ident_b)
                rq_rows = rq_pool.tile([4, 2, P], BF16, tag="rqrow", name="rq_rows")
                nc.vector.tensor_copy(rq_rows, ps_r)
                wb_g = wb_pool.tile([P, 4, 2, P], BF16, tag="wb", name="wb_g")
                st["wb"][g] = wb_g
                for jj in range(4):
                    _bcast_dma(nc, wb_g[:, jj, :, :], rq_rows[jj : jj + 1, :, :])

                # attnV evacuation (raw), then transpose + normalize into outT
                nc.scalar.copy(
                    st["attnV_sb"][:, :, 4 * g : 4 * g + 4, :], pg[:, :, :, 0:64]
                )
                ps_t = ps_misc.tile([P, 2, 2, P], BF16, tag="misc", name="ps_ot")
                for pl in range(2):
                    pr = 2 * g + pl
                    for s2 in range(2):
                        nc.tensor.transpose(
                            ps_t[:, pl, s2, :],
                            st["attnV_sb"][:, s2, 2 * pr : 2 * pr + 2, :].rearrange(
                                "p h d -> p (h d)"
                            ),
                            ident_b,
                        )
                for pl in range(2):
                    pr = 2 * g + pl
                    for hh, lo in ((0, 0), (1, 64)):
                        nc.vector.scalar_tensor_tensor(
                            st["outT"][lo : lo + 64, pr, :].rearrange(
                                "p (a s) -> p a s", a=2
                            ),
                            ps_t[lo : lo + 64, pl, :, :],
                            HVAL,
                            wb_g[lo : lo + 64, 2 * pl + hh, :, :],
                            OP.mult,
                            OP.mult,
                        )

            def emit_groupavg(st, g):
                # avg-attn accumulation for the 4 heads of this group
                wb_g = st["wb"].pop(g)
                for jj in range(4):
                    h = 4 * g + jj
                    pr2, hh = divmod(h, 2)
                    eT_h = st["eT"][pr2][:, hh]
                    wb_b = (
                        wb_g[:, jj, :, :]
                        .rearrange("p a s -> p (a s)")[:, None, :]
                        .to_broadcast([P, TT, SC])
                    )
                    if h in POOL_HEADS:
                        eng, acc, first = nc.gpsimd, st["acc_p"], st["first_p"]
                    else:
                        eng, acc, first = nc.vector, st["acc_v"], st["first_v"]
                    if first[0]:
                        eng.tensor_tensor(acc, eT_h, wb_b, OP.mult)
                        first[0] = False
                    else:
                        tag = "tmp_p" if h in POOL_HEADS else "tmp_v"
                        tmp = acc_pool.tile(
                            [P, TT, SC], BF16, tag=tag, name="tmp_h", bufs=2
                        )
                        eng.tensor_tensor(tmp, eT_h, wb_b, OP.mult)
                        eng.tensor_tensor(acc, acc, tmp, OP.add)

            prev_st = None
            for c in range(NCH):
                s0 = c * SC
                x_sb = xload.tile([P, 2, E], BF16, tag="x_sb", name="x_sb")
                nc.sync.dma_start(
                    out=x_sb, in_=xb[s0 : s0 + SC, :].rearrange("(a p) e -> p a e", p=P)
                )
                xT_c = xt_pool.tile([P, KE, SC], BF16)
                for eh in range(2):  # halves of the e8 range
                    ps_x = ps_misc.tile([P, 4, 2, P], BF16, tag="misc", name="ps_xt")
                    for e4 in range(4):
                        e8 = eh * 4 + e4
                        for s2 in range(2):
                            nc.tensor.transpose(
                                ps_x[:, e4, s2, :],
                                x_sb[:, s2, e8 * P : (e8 + 1) * P],
                                ident_b,
                            )
                    nc.vector.tensor_copy(
                        xT_c[:, eh * 4 : (eh + 1) * 4, :].rearrange(
                            "p k (a s) -> p k a s", a=2
                        ),
                        ps_x,
                    )

                qT_c = qt_pool.tile([P, KE, SC], BF16)
                for mh in range(4):  # pairs of m8
                    ps_q = ps_misc.tile([P, 2, SC], F32, tag="misc", name="ps_q")
                    for i in range(2):
                        m8 = mh * 2 + i
                        for k8 in range(KE):
                            nc.tensor.matmul(
                                ps_q[:, i, :],
                                Wq_bf[:, k8, m8 * P : (m8 + 1) * P],
                                xT_c[:, k8, :],
                                start=(k8 == 0),
                                stop=(k8 == KE - 1),
                            )
                    for i in range(2):
                        m8 = mh * 2 + i
                        nc.vector.tensor_scalar(
                            qT_c[:, m8, :],
                            ps_q[:, i, :],
                            bq_sb[:, m8 : m8 + 1],
                            None,
                            OP.add,
                        )

                # deferred work of the previous chunk, interleaved for latency:
                # norms (small serial chains) first, projection next, bulk avg last
                if prev_st is not None:
                    emit_groupnorm(prev_st, 2)
                    emit_groupnorm(prev_st, 3)
                    emit_outproj(prev_st)
                    emit_groupavg(prev_st, 2)
                    emit_groupavg(prev_st, 3)

                st = {
                    "s0": s0,
                    "qT_c": qT_c,
                    "acc_v": acc_pool.tile([P, TT, SC], BF16, tag="accv", name="acc_v"),
                    "acc_p": acc_pool.tile([P, TT, SC], BF16, tag="accp", name="acc_p"),
                    "outT": ot_pool.tile([P, KE, SC], BF16, name="outT"),
                    "attnV_sb": att_pool.tile([P, 2, H, D], BF16, name="attnV_sb"),
                    "eT": {},
                    "ps_av": {},
                    "wb": {},
                    "first_v": [True],
                    "first_p": [True],
                }

                for pr in range(H // 2):
                    emit_scores_half(st, pr, 0)
                    if pr >= 1:
                        emit_attnv_units(st, pr - 1, (0, 1))
                    emit_scores_half(st, pr, 1)
                    if pr >= 1:
                        emit_attnv_units(st, pr - 1, (2, 3))
                    if pr == 1 and prev_st is not None:
                        emit_avgout(prev_st)
                    if pr in (3, 5):
                        g = (pr - 3) // 2
                        emit_groupnorm(st, g)
                        emit_groupavg(st, g)
                emit_attnv_units(st, H // 2 - 1, (0, 1, 2, 3))
                prev_st = st

            emit_groupnorm(prev_st, 2)
            emit_groupnorm(prev_st, 3)
            emit_outproj(prev_st)
            emit_groupavg(prev_st, 2)
            emit_groupavg(prev_st, 3)
            emit_avgout(prev_st)


def get_program():
    global _PROGRAM
    if _PROGRAM is None:
        _PROGRAM = build_program()
    return _PROGRAM


def prep_inputs(inputs):
    """Host-side prep: bf16 conversion + bo_eff fold. Returns per-core input maps."""
    import ml_dtypes

    bf16 = ml_dtypes.bfloat16
    f32 = np.float32
    Wo = np.asarray(inputs["Wo"], dtype=f32)
    boe = np.asarray(inputs["bv"], dtype=f32) @ Wo + np.asarray(inputs["bo"], dtype=f32)
    common = {
        "Wq": np.ascontiguousarray(np.asarray(inputs["Wq"], dtype=f32).astype(bf16)),
        "Wk": np.ascontiguousarray(np.asarray(inputs["Wk"], dtype=f32).astype(bf16)),
        "Wv": np.ascontiguousarray(np.asarray(inputs["Wv"], dtype=f32).astype(bf16)),
        "Wo": np.ascontiguousarray(Wo.astype(bf16)),
        "bq": np.ascontiguousarray(np.asarray(inputs["bq"], dtype=f32)),
        "bk": np.ascontiguousarray(np.asarray(inputs["bk"], dtype=f32)),
        "boe": np.ascontiguousarray(boe.astype(bf16)),
    }
    x = np.asarray(inputs["x"], dtype=f32).astype(bf16)
    enc = np.asarray(inputs["encoder_output"], dtype=f32).astype(bf16)
    return [
        dict(common, x=np.ascontiguousarray(x[b]), enc=np.ascontiguousarray(enc[b]))
        for b in range(N_CORES)
    ]


def kernel(**inputs):
    nc = get_program()
    in_maps = prep_inputs(inputs)
    res = run_bass_kernel_spmd(nc, in_maps, list(range(N_CORES)))
    out = np.stack(
        [np.asarray(res.results[b]["out"]).astype(np.float32) for b in range(N_CORES)]
    )
    avg = np.stack(
        [np.asarray(res.results[b]["avg"]).astype(np.float32) for b in range(N_CORES)]
    )
    return out, avg


# revision 27
# speedup vs baseline: 1.2466x; 1.0578x over previous
"""Trainium2 Bass kernel for DecoderCrossAttention (B=8, S=2048, T=1024, E=1024, C=768, H=16, D=64).

Data-parallel over batch: 8 NeuronCores, one batch element each, no collectives.

v2 design (vs v1 baseline):
  - host passes x/enc/weights pre-converted to bf16 (no on-device converts, half the DMA)
  - host precomputes bo_eff = bv @ Wo + bo
  - attn@V flipped to out[s,d] = sum_t eT[t,s] v[t,d] (M=128, half the PE columns) with a
    x16 ones-column appended to v -> softmax row sums fall out of the same matmuls
    (kills the old M=32 ones-matmul pass entirely); 16 == H so the reciprocal is
    directly the avg-attn weight r_h[s]/H
  - recip columns -> rows via tiny PE transposes; one grouped wb broadcast per 4 heads
  - attnV transposed back to [e',s] on PE, normalized during evacuation via one STT
    pass (x16 compensation folded into the STT scalar)
  - avg-attn accumulation split DVE (12 heads) / GpSimd-Pool (4 heads)
  - outputs written bf16; host converts to f32
"""

import sys

sys.path.insert(0, "/opt/trn_rl_repo")

from contextlib import ExitStack

import numpy as np

import concourse.bass as bass
import concourse.mybir as mybir
import concourse.tile as tile
from concourse import bacc
from concourse.bass_utils import run_bass_kernel_spmd
from concourse.masks import make_identity

F32 = mybir.dt.float32
BF16 = mybir.dt.bfloat16
AF = mybir.ActivationFunctionType
OP = mybir.AluOpType

N_CORES = 8
S, T, E, C = 2048, 1024, 1024, 768
H, D = 16, 64
P = 128
SC = 256  # S-chunk size
NCH = S // SC
KE = E // P  # 8
KC = C // P  # 6
TT = T // P  # 8
SCALE = 0.125
HVAL = 16.0  # ones-column value == H: row sums arrive pre-scaled by H
POOL_HEADS = (0, 3, 6, 9, 12)  # avg-attn heads accumulated on GpSimd instead of DVE

_PROGRAM = None


def _bcast_dma(nc, out_t, src_row):
    """Broadcast a [1, F...] SBUF row to out_t [rows, F...] via zero-step DMA."""
    src_b = bass.AP(
        tensor=src_row.tensor,
        offset=src_row.offset,
        ap=[list(src_row.ap[0]), [0, out_t.shape[0]]]
        + [list(d) for d in src_row.ap[1:]],
    )
    nc.sync.dma_start(out=out_t, in_=src_b)


def build_program(loop_iters=0):
    nc = bacc.Bacc("TRN2", target_bir_lowering=False, debug=False, num_devices=N_CORES)

    xb = nc.dram_tensor("x", [S, E], BF16, kind="ExternalInput").ap()
    encb = nc.dram_tensor("enc", [T, C], BF16, kind="ExternalInput").ap()
    Wq = nc.dram_tensor("Wq", [E, E], BF16, kind="ExternalInput").ap()
    Wk = nc.dram_tensor("Wk", [C, E], BF16, kind="ExternalInput").ap()
    Wv = nc.dram_tensor("Wv", [C, E], BF16, kind="ExternalInput").ap()
    Wo = nc.dram_tensor("Wo", [E, E], BF16, kind="ExternalInput").ap()
    bq = nc.dram_tensor("bq", [E], F32, kind="ExternalInput").ap()
    bk = nc.dram_tensor("bk", [E], F32, kind="ExternalInput").ap()
    boe = nc.dram_tensor("boe", [E], BF16, kind="ExternalInput").ap()
    out = nc.dram_tensor("out", [S, E], BF16, kind="ExternalOutput").ap()
    avg = nc.dram_tensor("avg", [S, T], BF16, kind="ExternalOutput").ap()

    with tile.TileContext(nc) as tc:
        if loop_iters:
            with tc.For_i(0, loop_iters, 1):
                _build(tc, xb, encb, Wq, Wk, Wv, Wo, bq, bk, boe, out, avg)
        else:
            _build(tc, xb, encb, Wq, Wk, Wv, Wo, bq, bk, boe, out, avg)
    nc.compile()
    return nc


def _build(tc, xb, encb, Wq, Wk, Wv, Wo, bq, bk, boe, out, avg):
    nc = tc.nc
    with ExitStack() as stack:
        consts = stack.enter_context(tc.tile_pool(name="consts", bufs=1))
        resident = stack.enter_context(tc.tile_pool(name="resident", bufs=1))

        ident_b = consts.tile([P, P], BF16)
        make_identity(nc, ident_b)
        ident_f = consts.tile([P, P], F32)
        make_identity(nc, ident_f)
        ones_row = consts.tile([1, P], BF16)
        nc.vector.memset(ones_row, 1.0)
        bq_sb = consts.tile([P, KE], F32)
        nc.sync.dma_start(out=bq_sb, in_=bq.rearrange("(m p) -> p m", p=P))
        bk_sb = consts.tile([P, KE], F32)
        nc.sync.dma_start(out=bk_sb, in_=bk.rearrange("(m p) -> p m", p=P))
        boe_row = consts.tile([1, E], BF16)
        nc.sync.dma_start(out=boe_row, in_=boe[None, :])

        Wq_bf = resident.tile([P, KE, E], BF16)
        Wo_bf = resident.tile([P, KE, E], BF16)
        kT_bf = resident.tile([P, KE, T], BF16)
        v0x = resident.tile([P, TT, H, 65], BF16)  # [t, t8, head, d | sum-col]
        nc.vector.memset(v0x[:, :, :, 64:65], HVAL)

        # ---------------- phase A: K/V projections ----------------
        with ExitStack() as ph:
            early = ph.enter_context(tc.tile_pool(name="early", bufs=1))
            ld_pool = ph.enter_context(tc.tile_pool(name="ld_pool", bufs=3))
            ph_ps = ph.enter_context(tc.tile_pool(name="ph_ps", bufs=2, space="PSUM"))
            ph_tr = ph.enter_context(tc.tile_pool(name="ph_tr", bufs=2, space="PSUM"))

            # DMA order matters: enc tiles feed the first PE ops; big weight
            # loads go after the tensors needed soonest.
            enc_tiles = []
            for t8 in range(TT):
                enc_t = ld_pool.tile([P, C], BF16, tag="ld", name="enc_t", bufs=8)
                nc.sync.dma_start(out=enc_t, in_=encb[t8 * P : (t8 + 1) * P, :])
                enc_tiles.append(enc_t)
            Wk_bf = early.tile([P, KC, E], BF16)
            nc.sync.dma_start(out=Wk_bf, in_=Wk.rearrange("(k p) e -> p k e", p=P))
            Wv_bf = early.tile([P, KC, E], BF16)
            nc.sync.dma_start(out=Wv_bf, in_=Wv.rearrange("(k p) e -> p k e", p=P))
            nc.sync.dma_start(out=Wq_bf, in_=Wq.rearrange("(k p) e -> p k e", p=P))
            nc.sync.dma_start(out=Wo_bf, in_=Wo.rearrange("(k p) e -> p k e", p=P))
            encT_bf = early.tile([P, KC, T], BF16)

            # enc -> PE-transpose -> encT
            for t8 in range(TT):
                enc_t = enc_tiles[t8]
                ps_a = ph_tr.tile([P, C], BF16, tag="phb", name="ps_tr")
                for c6 in range(KC):
                    nc.tensor.transpose(
                        ps_a[:, c6 * P : (c6 + 1) * P],
                        enc_t[:, c6 * P : (c6 + 1) * P],
                        ident_b,
                    )
                nc.vector.tensor_copy(
                    encT_bf[:, :, t8 * P : (t8 + 1) * P],
                    ps_a.rearrange("p (k t) -> p k t", k=KC),
                )

            # kT[e',t] = sum_c Wk[c,e'] encT[c,t], +bk
            for m8 in range(KE):
                ps_a = ph_ps.tile([P, T], F32, tag="ph", name="ps_k")
                for c6 in range(KC):
                    for n2 in range(2):
                        nc.tensor.matmul(
                            ps_a[:, n2 * 512 : (n2 + 1) * 512],
                            Wk_bf[:, c6, m8 * P : (m8 + 1) * P],
                            encT_bf[:, c6, n2 * 512 : (n2 + 1) * 512],
                            start=(c6 == 0),
                            stop=(c6 == KC - 1),
                        )
                nc.scalar.activation(
                    kT_bf[:, m8, :], ps_a, AF.Identity, bias=bk_sb[:, m8 : m8 + 1]
                )

            # v0[t,e'] = sum_c encT[c,t] Wv[c,e']  (bv folded into boe on host)
            for t8 in range(TT):
                ps_a = ph_ps.tile([P, T], F32, tag="ph", name="ps_v")
                for c6 in range(KC):
                    for n2 in range(2):
                        nc.tensor.matmul(
                            ps_a[:, n2 * 512 : (n2 + 1) * 512],
                            encT_bf[:, c6, t8 * P : (t8 + 1) * P],
                            Wv_bf[:, c6, n2 * 512 : (n2 + 1) * 512],
                            start=(c6 == 0),
                            stop=(c6 == KC - 1),
                        )
                nc.vector.tensor_copy(
                    v0x[:, t8, :, 0:64],
                    ps_a.rearrange("p (h d) -> p h d", h=H),
                )

        # ---------------- phase B: S-chunk loop ----------------
        with ExitStack() as mn:
            xload = mn.enter_context(tc.tile_pool(name="xload", bufs=2))
            xt_pool = mn.enter_context(tc.tile_pool(name="xt_pool", bufs=2))
            qt_pool = mn.enter_context(tc.tile_pool(name="qt_pool", bufs=2))
            et_pool = mn.enter_context(tc.tile_pool(name="et_pool", bufs=6))
            wb_pool = mn.enter_context(tc.tile_pool(name="wb_pool", bufs=3))
            rq_pool = mn.enter_context(tc.tile_pool(name="rq_pool", bufs=3))
            att_pool = mn.enter_context(tc.tile_pool(name="att_pool", bufs=2))
            acc_pool = mn.enter_context(tc.tile_pool(name="acc_pool", bufs=2))
            ot_pool = mn.enter_context(tc.tile_pool(name="ot_pool", bufs=2))
            sb_pool = mn.enter_context(tc.tile_pool(name="sb_pool", bufs=2))

            ps_scores = mn.enter_context(
                tc.tile_pool(name="ps_scores", bufs=2, space="PSUM")
            )
            ps_avp = mn.enter_context(tc.tile_pool(name="ps_avp", bufs=1, space="PSUM"))
            ps_misc = mn.enter_context(tc.tile_pool(name="ps_misc", bufs=2, space="PSUM"))

            def emit_outproj(st):
                s0p, outT_p = st["s0"], st["outT"]
                # final projection: out[s,e] = outT^T @ Wo + boe
                for m2 in range(2):
                    out_sb = sb_pool.tile([P, E], BF16, tag="out_sb", name="out_sb")
                    for n2 in range(2):
                        ps_m = ps_misc.tile([P, 512], F32, tag="misc", name="ps_o")
                        for k8 in range(KE):
                            nc.tensor.matmul(
                                ps_m,
                                outT_p[:, k8, m2 * P : (m2 + 1) * P],
                                Wo_bf[:, k8, n2 * 512 : (n2 + 1) * 512],
                                start=(k8 == 0),
                                stop=False,
                            )
                        nc.tensor.matmul(
                            ps_m,
                            ones_row,
                            boe_row[:, n2 * 512 : (n2 + 1) * 512],
                            start=False,
                            stop=True,
                        )
                        if n2 == 0:
                            nc.scalar.copy(out_sb[:, n2 * 512 : (n2 + 1) * 512], ps_m)
                        else:
                            nc.vector.tensor_copy(
                                out_sb[:, n2 * 512 : (n2 + 1) * 512], ps_m
                            )
                    nc.sync.dma_start(
                        out=out[s0p + m2 * P : s0p + (m2 + 1) * P, :], in_=out_sb
                    )

            def emit_avgout(st):
                s0p, acc_v_p, acc_p_p = st["s0"], st["acc_v"], st["acc_p"]
                # avg_attn: merge acc halves, PE-transpose back to [s, t]
                acc_m = acc_pool.tile([P, TT, SC], BF16, tag="accm", name="acc_m", bufs=1)
                nc.vector.tensor_tensor(acc_m, acc_v_p, acc_p_p, OP.add)
                for s2 in range(2):
                    ps_m = ps_misc.tile([P, T], BF16, tag="misc", name="ps_av_t")
                    for t8 in range(TT):
                        nc.tensor.transpose(
                            ps_m[:, t8 * P : (t8 + 1) * P],
                            acc_m[:, t8, s2 * P : (s2 + 1) * P],
                            ident_b,
                        )
                    avg_sb = sb_pool.tile([P, T], BF16, tag="avg_sb", name="avg_sb")
                    nc.scalar.copy(avg_sb, ps_m)
                    nc.sync.dma_start(
                        out=avg[s0p + s2 * P : s0p + (s2 + 1) * P, :], in_=avg_sb
                    )

            def emit_scores_half(st, pr, half):
                if half == 0:
                    st["eT"][pr] = et_pool.tile(
                        [P, 2, TT, SC], BF16, tag="eT", name="eT_pair"
                    )
                ps_sc = {
                    0: ps_scores.tile([P, 4, SC], F32, tag="scores", name="ps_sc_a"),
                    1: ps_scores.tile([P, 4, SC], F32, tag="scores", name="ps_sc_b"),
                }
                for t4 in range(4):
                    t8 = half * 4 + t4
                    for hh, tp in ((0, 0), (1, 64)):
                        nc.tensor.matmul(
                            ps_sc[hh][:, t4, :],
                            kT_bf[tp : tp + D, pr, t8 * P : (t8 + 1) * P],
                            st["qT_c"][tp : tp + D, pr, :],
                            start=True,
                            stop=True,
                            tile_position=(tp, 0),
                        )
                for hh in range(2):
                    nc.scalar.activation(
                        st["eT"][pr][:, hh, half * 4 : half * 4 + 4, :],
                        ps_sc[hh],
                        AF.Exp,
                        scale=SCALE,
                    )

            def emit_attnv_units(st, pr, units):
                g, pl = divmod(pr, 2)
                for u in units:
                    hh, s2 = divmod(u, 2)
                    if u == 0 and pl == 0:
                        st["ps_av"][g] = ps_avp.tile(
                            [P, 2, 4, P], F32, tag="po", name="ps_av"
                        )
                    h = 2 * pr + hh
                    j = 2 * pl + hh
                    for t8 in range(TT):
                        nc.tensor.matmul(
                            st["ps_av"][g][:, s2, j, 0:65],
                            st["eT"][pr][:, hh, t8, s2 * P : (s2 + 1) * P],
                            v0x[:, t8, h, :],
                            start=(t8 == 0),
                            stop=(t8 == TT - 1),
                        )

            def emit_groupnorm(st, g):
                # reciprocal of the x16 row sums -> r_h[s]/16 == r_h[s]/H
                pg = st["ps_av"].pop(g)
                rqr = rq_pool.tile([P, 2, 4], F32, tag="rqr", name="rqr")
                nc.vector.reciprocal_approx_fast(
                    out=rqr,
                    in_=pg[:, :, :, 64:65].rearrange("p a b c -> p a (b c)"),
                )
                # column -> row transposes (f32), then per-head broadcasts
                ps_r = ps_misc.tile([4, 2, P], F32, tag="misc", name="ps_rq")
                for s2 in range(2):
                    nc.tensor.transpose(ps_r[:, s2, :], rqr[:, s2, :], ident_f)
                rq_rows = rq_pool.tile([4, 2, P], BF16, tag="rqrow", name="rq_rows")
                nc.vector.tensor_copy(rq_rows, ps_r)
                wb_g = wb_pool.tile([P, 4, 2, P], BF16, tag="wb", name="wb_g")
                st["wb"][g] = wb_g
                for jj in range(4):
                    _bcast_dma(nc, wb_g[:, jj, :, :], rq_rows[jj : jj + 1, :, :])

                # attnV evacuation (raw), then transpose + normalize into outT
                nc.scalar.copy(
                    st["attnV_sb"][:, :, 4 * g : 4 * g + 4, :], pg[:, :, :, 0:64]
                )
                ps_t = ps_misc.tile([P, 2, 2, P], BF16, tag="misc", name="ps_ot")
                for pl in range(2):
                    pr = 2 * g + pl
                    for s2 in range(2):
                        nc.tensor.transpose(
                            ps_t[:, pl, s2, :],
                            st["attnV_sb"][:, s2, 2 * pr : 2 * pr + 2, :].rearrange(
                                "p h d -> p (h d)"
                            ),
                            ident_b,
                        )
                for pl in range(2):
                    pr = 2 * g + pl
                    for hh, lo in ((0, 0), (1, 64)):
                        nc.vector.scalar_tensor_tensor(
                            st["outT"][lo : lo + 64, pr, :].rearrange(
                                "p (a s) -> p a s", a=2
                            ),
                            ps_t[lo : lo + 64, pl, :, :],
                            HVAL,
                            wb_g[lo : lo + 64, 2 * pl + hh, :, :],
                            OP.mult,
                            OP.mult,
                        )

            def emit_groupavg(st, g):
                # avg-attn accumulation for the 4 heads of this group
                wb_g = st["wb"].pop(g)
                for jj in range(4):
                    h = 4 * g + jj
                    pr2, hh = divmod(h, 2)
                    eT_h = st["eT"][pr2][:, hh]
                    wb_b = (
                        wb_g[:, jj, :, :]
                        .rearrange("p a s -> p (a s)")[:, None, :]
                        .to_broadcast([P, TT, SC])
                    )
                    if h in POOL_HEADS:
                        eng, acc, first = nc.gpsimd, st["acc_p"], st["first_p"]
                    else:
                        eng, acc, first = nc.vector, st["acc_v"], st["first_v"]
                    if first[0]:
                        eng.tensor_tensor(acc, eT_h, wb_b, OP.mult)
                        first[0] = False
                    else:
                        tag = "tmp_p" if h in POOL_HEADS else "tmp_v"
                        tmp = acc_pool.tile(
                            [P, TT, SC], BF16, tag=tag, name="tmp_h", bufs=2
                        )
                        eng.tensor_tensor(tmp, eT_h, wb_b, OP.mult)
                        eng.tensor_tensor(acc, acc, tmp, OP.add)

            prev_st = None
            for c in range(NCH):
                s0 = c * SC
                x_sb = xload.tile([P, 2, E], BF16, tag="x_sb", name="x_sb")
                nc.sync.dma_start(
                    out=x_sb, in_=xb[s0 : s0 + SC, :].rearrange("(a p) e -> p a e", p=P)
                )
                xT_c = xt_pool.tile([P, KE, SC], BF16)
                for eh in range(2):  # halves of the e8 range
                    ps_x = ps_misc.tile([P, 4, 2, P], BF16, tag="misc", name="ps_xt")
                    for e4 in range(4):
                        e8 = eh * 4 + e4
                        for s2 in range(2):
                            nc.tensor.transpose(
                                ps_x[:, e4, s2, :],
                                x_sb[:, s2, e8 * P : (e8 + 1) * P],
                                ident_b,
                            )
                    nc.vector.tensor_copy(
                        xT_c[:, eh * 4 : (eh + 1) * 4, :].rearrange(
                            "p k (a s) -> p k a s", a=2
                        ),
                        ps_x,
                    )

                qT_c = qt_pool.tile([P, KE, SC], BF16)
                for mh in range(4):  # pairs of m8
                    ps_q = ps_misc.tile([P, 2, SC], F32, tag="misc", name="ps_q")
                    for i in range(2):
                        m8 = mh * 2 + i
                        for k8 in range(KE):
                            nc.tensor.matmul(
                                ps_q[:, i, :],
                                Wq_bf[:, k8, m8 * P : (m8 + 1) * P],
                                xT_c[:, k8, :],
                                start=(k8 == 0),
                                stop=(k8 == KE - 1),
                            )
                    for i in range(2):
                        m8 = mh * 2 + i
                        nc.vector.tensor_scalar(
                            qT_c[:, m8, :],
                            ps_q[:, i, :],
                            bq_sb[:, m8 : m8 + 1],
                            None,
                            OP.add,
                        )

                # deferred work of the previous chunk, interleaved for latency:
                # norms (small serial chains) first, projection next, bulk avg last
                if prev_st is not None:
                    emit_groupnorm(prev_st, 2)
                    emit_groupnorm(prev_st, 3)
                    emit_outproj(prev_st)
                    emit_groupavg(prev_st, 2)
                    emit_groupavg(prev_st, 3)

                st = {
                    "s0": s0,
                    "qT_c": qT_c,
                    "acc_v": acc_pool.tile([P, TT, SC], BF16, tag="accv", name="acc_v"),
                    "acc_p": acc_pool.tile([P, TT, SC], BF16, tag="accp", name="acc_p"),
                    "outT": ot_pool.tile([P, KE, SC], BF16, name="outT"),
                    "attnV_sb": att_pool.tile([P, 2, H, D], BF16, name="attnV_sb"),
                    "eT": {},
                    "ps_av": {},
                    "wb": {},
                    "first_v": [True],
                    "first_p": [True],
                }

                for pr in range(H // 2):
                    emit_scores_half(st, pr, 0)
                    if pr >= 1:
                        emit_attnv_units(st, pr - 1, (0, 1))
                    emit_scores_half(st, pr, 1)
                    if pr >= 1:
                        emit_attnv_units(st, pr - 1, (2, 3))
                    if pr == 1 and prev_st is not None:
                        emit_avgout(prev_st)
                    if pr in (3, 5):
                        g = (pr - 3) // 2
                        emit_groupnorm(st, g)
                        emit_groupavg(st, g)
                emit_attnv_units(st, H // 2 - 1, (0, 1, 2, 3))
                prev_st = st

            emit_groupnorm(prev_st, 2)
            emit_groupnorm(prev_st, 3)
            emit_outproj(prev_st)
            emit_groupavg(prev_st, 2)
            emit_groupavg(prev_st, 3)
            emit_avgout(prev_st)


def get_program():
    global _PROGRAM
    if _PROGRAM is None:
        _PROGRAM = build_program()
    return _PROGRAM


def prep_inputs(inputs):
    """Host-side prep: bf16 conversion + bo_eff fold. Returns per-core input maps."""
    import ml_dtypes

    bf16 = ml_dtypes.bfloat16
    f32 = np.float32
    Wo = np.asarray(inputs["Wo"], dtype=f32)
    boe = np.asarray(inputs["bv"], dtype=f32) @ Wo + np.asarray(inputs["bo"], dtype=f32)
    common = {
        "Wq": np.ascontiguousarray(np.asarray(inputs["Wq"], dtype=f32).astype(bf16)),
        "Wk": np.ascontiguousarray(np.asarray(inputs["Wk"], dtype=f32).astype(bf16)),
        "Wv": np.ascontiguousarray(np.asarray(inputs["Wv"], dtype=f32).astype(bf16)),
        "Wo": np.ascontiguousarray(Wo.astype(bf16)),
        "bq": np.ascontiguousarray(np.asarray(inputs["bq"], dtype=f32)),
        "bk": np.ascontiguousarray(np.asarray(inputs["bk"], dtype=f32)),
        "boe": np.ascontiguousarray(boe.astype(bf16)),
    }
    x = np.asarray(inputs["x"], dtype=f32).astype(bf16)
    enc = np.asarray(inputs["encoder_output"], dtype=f32).astype(bf16)
    return [
        dict(common, x=np.ascontiguousarray(x[b]), enc=np.ascontiguousarray(enc[b]))
        for b in range(N_CORES)
    ]


def kernel(**inputs):
    nc = get_program()
    in_maps = prep_inputs(inputs)
    res = run_bass_kernel_spmd(nc, in_maps, list(range(N_CORES)))
    out = np.stack(
        [np.asarray(res.results[b]["out"]).astype(np.float32) for b in range(N_CORES)]
    )
    avg = np.stack(
        [np.asarray(res.results[b]["avg"]).astype(np.float32) for b in range(N_CORES)]
    )
    return out, avg


# revision 32
# speedup vs baseline: 2.4634x; 1.9760x over previous
"""Trainium2 Bass kernel for DecoderCrossAttention (B=8, S=2048, T=1024, E=1024, C=768, H=16, D=64).

Data-parallel over batch: 8 NeuronCores, one batch element each, no collectives.

v2 design (vs v1 baseline):
  - host passes x/enc/weights pre-converted to bf16 (no on-device converts, half the DMA)
  - host precomputes bo_eff = bv @ Wo + bo
  - attn@V flipped to out[s,d] = sum_t eT[t,s] v[t,d] (M=128, half the PE columns) with a
    x16 ones-column appended to v -> softmax row sums fall out of the same matmuls
    (kills the old M=32 ones-matmul pass entirely); 16 == H so the reciprocal is
    directly the avg-attn weight r_h[s]/H
  - recip columns -> rows via tiny PE transposes; one grouped wb broadcast per 4 heads
  - attnV transposed back to [e',s] on PE, normalized during evacuation via one STT
    pass (x16 compensation folded into the STT scalar)
  - avg-attn accumulation split DVE (12 heads) / GpSimd-Pool (4 heads)
  - outputs written bf16; host converts to f32
"""

import sys

sys.path.insert(0, "/opt/trn_rl_repo")

from contextlib import ExitStack

import numpy as np

import concourse.bass as bass
import concourse.mybir as mybir
import concourse.tile as tile
from concourse import bacc
from concourse.bass_utils import run_bass_kernel_spmd
from concourse.masks import make_identity

F32 = mybir.dt.float32
BF16 = mybir.dt.bfloat16
AF = mybir.ActivationFunctionType
OP = mybir.AluOpType

N_CORES = 8
S, T, E, C = 2048, 1024, 1024, 768
H, D = 16, 64
P = 128
SC = 256  # S-chunk size
NCH = S // SC
KE = E // P  # 8
KC = C // P  # 6
TT = T // P  # 8
SCALE = 0.125
HVAL = 16.0  # ones-column value == H: row sums arrive pre-scaled by H
POOL_HEADS = (1, 4, 7, 10, 13)  # avg-attn heads accumulated on GpSimd instead of DVE

_PROGRAM = None


def _bcast_dma(nc, out_t, src_row):
    """Broadcast a [1, F...] SBUF row to out_t [rows, F...] via zero-step DMA."""
    src_b = bass.AP(
        tensor=src_row.tensor,
        offset=src_row.offset,
        ap=[list(src_row.ap[0]), [0, out_t.shape[0]]]
        + [list(d) for d in src_row.ap[1:]],
    )
    nc.sync.dma_start(out=out_t, in_=src_b)


def build_program(loop_iters=0):
    nc = bacc.Bacc("TRN2", target_bir_lowering=False, debug=False, num_devices=N_CORES)

    xb = nc.dram_tensor("x", [S, E], BF16, kind="ExternalInput").ap()
    encb = nc.dram_tensor("enc", [T, C], BF16, kind="ExternalInput").ap()
    Wq = nc.dram_tensor("Wq", [E, E], BF16, kind="ExternalInput").ap()
    Wk = nc.dram_tensor("Wk", [C, E], BF16, kind="ExternalInput").ap()
    Wv = nc.dram_tensor("Wv", [C, E], BF16, kind="ExternalInput").ap()
    Wo = nc.dram_tensor("Wo", [E, E], BF16, kind="ExternalInput").ap()
    bq = nc.dram_tensor("bq", [E], F32, kind="ExternalInput").ap()
    bk = nc.dram_tensor("bk", [E], F32, kind="ExternalInput").ap()
    boe = nc.dram_tensor("boe", [E], BF16, kind="ExternalInput").ap()
    out = nc.dram_tensor("out", [S, E], BF16, kind="ExternalOutput").ap()
    avg = nc.dram_tensor("avg", [S, T], BF16, kind="ExternalOutput").ap()

    with tile.TileContext(nc) as tc:
        if loop_iters:
            with tc.For_i(0, loop_iters, 1):
                _build(tc, xb, encb, Wq, Wk, Wv, Wo, bq, bk, boe, out, avg)
        else:
            _build(tc, xb, encb, Wq, Wk, Wv, Wo, bq, bk, boe, out, avg)
    nc.compile()
    return nc


def _build(tc, xb, encb, Wq, Wk, Wv, Wo, bq, bk, boe, out, avg):
    nc = tc.nc
    with ExitStack() as stack:
        consts = stack.enter_context(tc.tile_pool(name="consts", bufs=1))
        resident = stack.enter_context(tc.tile_pool(name="resident", bufs=1))

        ident_b = consts.tile([P, P], BF16)
        make_identity(nc, ident_b)
        ident_f = consts.tile([P, P], F32)
        make_identity(nc, ident_f)
        ones_row = consts.tile([1, P], BF16)
        nc.vector.memset(ones_row, 1.0)
        bq_sb = consts.tile([P, KE], F32)
        nc.sync.dma_start(out=bq_sb, in_=bq.rearrange("(m p) -> p m", p=P))
        bk_sb = consts.tile([P, KE], F32)
        nc.sync.dma_start(out=bk_sb, in_=bk.rearrange("(m p) -> p m", p=P))
        boe_row = consts.tile([1, E], BF16)
        nc.sync.dma_start(out=boe_row, in_=boe[None, :])

        Wq_bf = resident.tile([P, KE, E], BF16)
        Wo_bf = resident.tile([P, KE, E], BF16)
        kT_bf = resident.tile([P, KE, T], BF16)
        v0x = resident.tile([P, TT, H, 65], BF16)  # [t, t8, head, d | sum-col]
        nc.vector.memset(v0x[:, :, :, 64:65], HVAL)

        # ---------------- phase A: K/V projections ----------------
        with ExitStack() as ph:
            early = ph.enter_context(tc.tile_pool(name="early", bufs=1))
            ld_pool = ph.enter_context(tc.tile_pool(name="ld_pool", bufs=3))
            ph_ps = ph.enter_context(tc.tile_pool(name="ph_ps", bufs=2, space="PSUM"))
            ph_tr = ph.enter_context(tc.tile_pool(name="ph_tr", bufs=2, space="PSUM"))

            # DMA order matters: enc tiles feed the first PE ops; big weight
            # loads go after the tensors needed soonest.
            enc_tiles = []
            for t8 in range(TT):
                enc_t = ld_pool.tile([P, C], BF16, tag="ld", name="enc_t", bufs=8)
                nc.sync.dma_start(out=enc_t, in_=encb[t8 * P : (t8 + 1) * P, :])
                enc_tiles.append(enc_t)
            Wk_bf = early.tile([P, KC, E], BF16)
            nc.sync.dma_start(out=Wk_bf, in_=Wk.rearrange("(k p) e -> p k e", p=P))
            Wv_bf = early.tile([P, KC, E], BF16)
            nc.sync.dma_start(out=Wv_bf, in_=Wv.rearrange("(k p) e -> p k e", p=P))
            nc.sync.dma_start(out=Wq_bf, in_=Wq.rearrange("(k p) e -> p k e", p=P))
            nc.sync.dma_start(out=Wo_bf, in_=Wo.rearrange("(k p) e -> p k e", p=P))
            encT_bf = early.tile([P, KC, T], BF16)

            # enc -> PE-transpose -> encT
            for t8 in range(TT):
                enc_t = enc_tiles[t8]
                ps_a = ph_tr.tile([P, C], BF16, tag="phb", name="ps_tr")
                for c6 in range(KC):
                    nc.tensor.transpose(
                        ps_a[:, c6 * P : (c6 + 1) * P],
                        enc_t[:, c6 * P : (c6 + 1) * P],
                        ident_b,
                    )
                nc.vector.tensor_copy(
                    encT_bf[:, :, t8 * P : (t8 + 1) * P],
                    ps_a.rearrange("p (k t) -> p k t", k=KC),
                )

            # kT[e',t] = sum_c Wk[c,e'] encT[c,t], +bk
            for m8 in range(KE):
                ps_a = ph_ps.tile([P, T], F32, tag="ph", name="ps_k")
                for c6 in range(KC):
                    for n2 in range(2):
                        nc.tensor.matmul(
                            ps_a[:, n2 * 512 : (n2 + 1) * 512],
                            Wk_bf[:, c6, m8 * P : (m8 + 1) * P],
                            encT_bf[:, c6, n2 * 512 : (n2 + 1) * 512],
                            start=(c6 == 0),
                            stop=(c6 == KC - 1),
                        )
                nc.scalar.activation(
                    kT_bf[:, m8, :], ps_a, AF.Identity, bias=bk_sb[:, m8 : m8 + 1]
                )

            # v0[t,e'] = sum_c encT[c,t] Wv[c,e']  (bv folded into boe on host)
            for t8 in range(TT):
                ps_a = ph_ps.tile([P, T], F32, tag="ph", name="ps_v")
                for c6 in range(KC):
                    for n2 in range(2):
                        nc.tensor.matmul(
                            ps_a[:, n2 * 512 : (n2 + 1) * 512],
                            encT_bf[:, c6, t8 * P : (t8 + 1) * P],
                            Wv_bf[:, c6, n2 * 512 : (n2 + 1) * 512],
                            start=(c6 == 0),
                            stop=(c6 == KC - 1),
                        )
                nc.vector.tensor_copy(
                    v0x[:, t8, :, 0:64],
                    ps_a.rearrange("p (h d) -> p h d", h=H),
                )

        # ---------------- phase B: S-chunk loop ----------------
        with ExitStack() as mn:
            xload = mn.enter_context(tc.tile_pool(name="xload", bufs=2))
            xt_pool = mn.enter_context(tc.tile_pool(name="xt_pool", bufs=2))
            qt_pool = mn.enter_context(tc.tile_pool(name="qt_pool", bufs=2))
            et_pool = mn.enter_context(tc.tile_pool(name="et_pool", bufs=6))
            wb_pool = mn.enter_context(tc.tile_pool(name="wb_pool", bufs=3))
            rq_pool = mn.enter_context(tc.tile_pool(name="rq_pool", bufs=3))
            att_pool = mn.enter_context(tc.tile_pool(name="att_pool", bufs=2))
            acc_pool = mn.enter_context(tc.tile_pool(name="acc_pool", bufs=2))
            ot_pool = mn.enter_context(tc.tile_pool(name="ot_pool", bufs=2))
            sb_pool = mn.enter_context(tc.tile_pool(name="sb_pool", bufs=2))

            ps_scores = mn.enter_context(
                tc.tile_pool(name="ps_scores", bufs=2, space="PSUM")
            )
            ps_avp = mn.enter_context(tc.tile_pool(name="ps_avp", bufs=1, space="PSUM"))
            ps_misc = mn.enter_context(tc.tile_pool(name="ps_misc", bufs=2, space="PSUM"))

            def emit_outproj(st):
                s0p, outT_p = st["s0"], st["outT"]
                # final projection: out[s,e] = outT^T @ Wo + boe
                for m2 in range(2):
                    out_sb = sb_pool.tile([P, E], BF16, tag="out_sb", name="out_sb")
                    for n2 in range(2):
                        ps_m = ps_misc.tile([P, 512], F32, tag="misc", name="ps_o")
                        for k8 in range(KE):
                            nc.tensor.matmul(
                                ps_m,
                                outT_p[:, k8, m2 * P : (m2 + 1) * P],
                                Wo_bf[:, k8, n2 * 512 : (n2 + 1) * 512],
                                start=(k8 == 0),
                                stop=False,
                            )
                        nc.tensor.matmul(
                            ps_m,
                            ones_row,
                            boe_row[:, n2 * 512 : (n2 + 1) * 512],
                            start=False,
                            stop=True,
                        )
                        if n2 == 0:
                            nc.scalar.copy(out_sb[:, n2 * 512 : (n2 + 1) * 512], ps_m)
                        else:
                            nc.vector.tensor_copy(
                                out_sb[:, n2 * 512 : (n2 + 1) * 512], ps_m
                            )
                    nc.sync.dma_start(
                        out=out[s0p + m2 * P : s0p + (m2 + 1) * P, :], in_=out_sb
                    )

            def emit_avgout(st):
                s0p, acc_v_p, acc_p_p = st["s0"], st["acc_v"], st["acc_p"]
                # avg_attn: merge acc halves, PE-transpose back to [s, t]
                acc_m = acc_pool.tile([P, TT, SC], BF16, tag="accm", name="acc_m", bufs=1)
                nc.vector.tensor_tensor(acc_m, acc_v_p, acc_p_p, OP.add)
                for s2 in range(2):
                    ps_m = ps_misc.tile([P, T], BF16, tag="misc", name="ps_av_t")
                    for t8 in range(TT):
                        nc.tensor.transpose(
                            ps_m[:, t8 * P : (t8 + 1) * P],
                            acc_m[:, t8, s2 * P : (s2 + 1) * P],
                            ident_b,
                        )
                    avg_sb = sb_pool.tile([P, T], BF16, tag="avg_sb", name="avg_sb")
                    nc.scalar.copy(avg_sb, ps_m)
                    nc.sync.dma_start(
                        out=avg[s0p + s2 * P : s0p + (s2 + 1) * P, :], in_=avg_sb
                    )

            def emit_scores_half(st, pr, half):
                if half == 0:
                    st["eT"][pr] = et_pool.tile(
                        [P, 2, TT, SC], BF16, tag="eT", name="eT_pair"
                    )
                ps_sc = {
                    0: ps_scores.tile([P, 4, SC], F32, tag="scores", name="ps_sc_a"),
                    1: ps_scores.tile([P, 4, SC], F32, tag="scores", name="ps_sc_b"),
                }
                for t4 in range(4):
                    t8 = half * 4 + t4
                    for hh, tp in ((0, 0), (1, 64)):
                        nc.tensor.matmul(
                            ps_sc[hh][:, t4, :],
                            kT_bf[tp : tp + D, pr, t8 * P : (t8 + 1) * P],
                            st["qT_c"][tp : tp + D, pr, :],
                            start=True,
                            stop=True,
                            tile_position=(tp, 0),
                        )
                for hh in range(2):
                    nc.scalar.activation(
                        st["eT"][pr][:, hh, half * 4 : half * 4 + 4, :],
                        ps_sc[hh],
                        AF.Exp,
                        scale=SCALE,
                    )

            def emit_attnv_units(st, pr, units):
                g, pl = divmod(pr, 2)
                for u in units:
                    hh, s2 = divmod(u, 2)
                    if u == 0 and pl == 0:
                        st["ps_av"][g] = ps_avp.tile(
                            [P, 2, 4, P], F32, tag="po", name="ps_av"
                        )
                    h = 2 * pr + hh
                    j = 2 * pl + hh
                    for t8 in range(TT):
                        nc.tensor.matmul(
                            st["ps_av"][g][:, s2, j, 0:65],
                            st["eT"][pr][:, hh, t8, s2 * P : (s2 + 1) * P],
                            v0x[:, t8, h, :],
                            start=(t8 == 0),
                            stop=(t8 == TT - 1),
                        )

            def emit_groupnorm(st, g):
                # reciprocal of the x16 row sums -> r_h[s]/16 == r_h[s]/H
                pg = st["ps_av"].pop(g)
                rqr = rq_pool.tile([P, 2, 4], F32, tag="rqr", name="rqr")
                nc.vector.reciprocal_approx_fast(
                    out=rqr,
                    in_=pg[:, :, :, 64:65].rearrange("p a b c -> p a (b c)"),
                )
                # column -> row transposes (f32), then per-head broadcasts
                ps_r = ps_misc.tile([4, 2, P], F32, tag="misc", name="ps_rq")
                for s2 in range(2):
                    nc.tensor.transpose(ps_r[:, s2, :], rqr[:, s2, :], ident_f)
                rq_rows = rq_pool.tile([4, 2, P], BF16, tag="rqrow", name="rq_rows")
                nc.vector.tensor_copy(rq_rows, ps_r)
                wb_g = wb_pool.tile([P, 4, 2, P], BF16, tag="wb", name="wb_g")
                st["wb"][g] = wb_g
                for jj in range(4):
                    _bcast_dma(nc, wb_g[:, jj, :, :], rq_rows[jj : jj + 1, :, :])

                # attnV evacuation (raw), then transpose + normalize into outT
                nc.scalar.copy(
                    st["attnV_sb"][:, :, 4 * g : 4 * g + 4, :], pg[:, :, :, 0:64]
                )
                ps_t = ps_misc.tile([P, 2, 2, P], BF16, tag="misc", name="ps_ot")
                for pl in range(2):
                    pr = 2 * g + pl
                    for s2 in range(2):
                        nc.tensor.transpose(
                            ps_t[:, pl, s2, :],
                            st["attnV_sb"][:, s2, 2 * pr : 2 * pr + 2, :].rearrange(
                                "p h d -> p (h d)"
                            ),
                            ident_b,
                        )
                for pl in range(2):
                    pr = 2 * g + pl
                    for hh, lo in ((0, 0), (1, 64)):
                        nc.vector.scalar_tensor_tensor(
                            st["outT"][lo : lo + 64, pr, :].rearrange(
                                "p (a s) -> p a s", a=2
                            ),
                            ps_t[lo : lo + 64, pl, :, :],
                            HVAL,
                            wb_g[lo : lo + 64, 2 * pl + hh, :, :],
                            OP.mult,
                            OP.mult,
                        )

            def emit_groupavg(st, g):
                # avg-attn accumulation for the 4 heads of this group
                wb_g = st["wb"].pop(g)
                for jj in range(4):
                    h = 4 * g + jj
                    pr2, hh = divmod(h, 2)
                    eT_h = st["eT"][pr2][:, hh]
                    wb_b = (
                        wb_g[:, jj, :, :]
                        .rearrange("p a s -> p (a s)")[:, None, :]
                        .to_broadcast([P, TT, SC])
                    )
                    if h in POOL_HEADS:
                        eng, acc, first = nc.gpsimd, st["acc_p"], st["first_p"]
                    else:
                        eng, acc, first = nc.vector, st["acc_v"], st["first_v"]
                    if first[0]:
                        eng.tensor_tensor(acc, eT_h, wb_b, OP.mult)
                        first[0] = False
                    else:
                        tag = "tmp_p" if h in POOL_HEADS else "tmp_v"
                        tmp = acc_pool.tile(
                            [P, TT, SC], BF16, tag=tag, name="tmp_h", bufs=2
                        )
                        eng.tensor_tensor(tmp, eT_h, wb_b, OP.mult)
                        eng.tensor_tensor(acc, acc, tmp, OP.add)

            prev_st = None
            for c in range(NCH):
                s0 = c * SC
                x_sb = xload.tile([P, 2, E], BF16, tag="x_sb", name="x_sb")
                nc.sync.dma_start(
                    out=x_sb, in_=xb[s0 : s0 + SC, :].rearrange("(a p) e -> p a e", p=P)
                )
                xT_c = xt_pool.tile([P, KE, SC], BF16)
                for eh in range(2):  # halves of the e8 range
                    ps_x = ps_misc.tile([P, 4, 2, P], BF16, tag="misc", name="ps_xt")
                    for e4 in range(4):
                        e8 = eh * 4 + e4
                        for s2 in range(2):
                            nc.tensor.transpose(
                                ps_x[:, e4, s2, :],
                                x_sb[:, s2, e8 * P : (e8 + 1) * P],
                                ident_b,
                            )
                    nc.vector.tensor_copy(
                        xT_c[:, eh * 4 : (eh + 1) * 4, :].rearrange(
                            "p k (a s) -> p k a s", a=2
                        ),
                        ps_x,
                    )

                qT_c = qt_pool.tile([P, KE, SC], BF16)
                for mh in range(4):  # pairs of m8
                    ps_q = ps_misc.tile([P, 2, SC], F32, tag="misc", name="ps_q")
                    for i in range(2):
                        m8 = mh * 2 + i
                        for k8 in range(KE):
                            nc.tensor.matmul(
                                ps_q[:, i, :],
                                Wq_bf[:, k8, m8 * P : (m8 + 1) * P],
                                xT_c[:, k8, :],
                                start=(k8 == 0),
                                stop=(k8 == KE - 1),
                            )
                    for i in range(2):
                        m8 = mh * 2 + i
                        nc.vector.tensor_scalar(
                            qT_c[:, m8, :],
                            ps_q[:, i, :],
                            bq_sb[:, m8 : m8 + 1],
                            None,
                            OP.add,
                        )

                # deferred work of the previous chunk, interleaved for latency:
                # norms (small serial chains) first, projection next, bulk avg last
                if prev_st is not None:
                    emit_groupnorm(prev_st, 2)
                    emit_groupnorm(prev_st, 3)
                    emit_outproj(prev_st)
                    emit_groupavg(prev_st, 2)
                    emit_groupavg(prev_st, 3)

                st = {
                    "s0": s0,
                    "qT_c": qT_c,
                    "acc_v": acc_pool.tile([P, TT, SC], BF16, tag="accv", name="acc_v"),
                    "acc_p": acc_pool.tile([P, TT, SC], BF16, tag="accp", name="acc_p"),
                    "outT": ot_pool.tile([P, KE, SC], BF16, name="outT"),
                    "attnV_sb": att_pool.tile([P, 2, H, D], BF16, name="attnV_sb"),
                    "eT": {},
                    "ps_av": {},
                    "wb": {},
                    "first_v": [True],
                    "first_p": [True],
                }

                for pr in range(H // 2):
                    emit_scores_half(st, pr, 0)
                    if pr >= 1:
                        emit_attnv_units(st, pr - 1, (0, 1))
                    emit_scores_half(st, pr, 1)
                    if pr >= 1:
                        emit_attnv_units(st, pr - 1, (2, 3))
                    if pr == 1 and prev_st is not None:
                        emit_avgout(prev_st)
                    if pr in (3, 5):
                        g = (pr - 3) // 2
                        emit_groupnorm(st, g)
                        emit_groupavg(st, g)
                emit_attnv_units(st, H // 2 - 1, (0, 1, 2, 3))
                prev_st = st

            emit_groupnorm(prev_st, 2)
            emit_groupnorm(prev_st, 3)
            emit_outproj(prev_st)
            emit_groupavg(prev_st, 2)
            emit_groupavg(prev_st, 3)
            emit_avgout(prev_st)


def get_program():
    global _PROGRAM
    if _PROGRAM is None:
        _PROGRAM = build_program()
    return _PROGRAM


def prep_inputs(inputs):
    """Host-side prep: bf16 conversion + bo_eff fold. Returns per-core input maps."""
    import ml_dtypes

    bf16 = ml_dtypes.bfloat16
    f32 = np.float32
    Wo = np.asarray(inputs["Wo"], dtype=f32)
    boe = np.asarray(inputs["bv"], dtype=f32) @ Wo + np.asarray(inputs["bo"], dtype=f32)
    common = {
        "Wq": np.ascontiguousarray(np.asarray(inputs["Wq"], dtype=f32).astype(bf16)),
        "Wk": np.ascontiguousarray(np.asarray(inputs["Wk"], dtype=f32).astype(bf16)),
        "Wv": np.ascontiguousarray(np.asarray(inputs["Wv"], dtype=f32).astype(bf16)),
        "Wo": np.ascontiguousarray(Wo.astype(bf16)),
        "bq": np.ascontiguousarray(np.asarray(inputs["bq"], dtype=f32)),
        "bk": np.ascontiguousarray(np.asarray(inputs["bk"], dtype=f32)),
        "boe": np.ascontiguousarray(boe.astype(bf16)),
    }
    x = np.asarray(inputs["x"], dtype=f32).astype(bf16)
    enc = np.asarray(inputs["encoder_output"], dtype=f32).astype(bf16)
    return [
        dict(common, x=np.ascontiguousarray(x[b]), enc=np.ascontiguousarray(enc[b]))
        for b in range(N_CORES)
    ]


def kernel(**inputs):
    nc = get_program()
    in_maps = prep_inputs(inputs)
    res = run_bass_kernel_spmd(nc, in_maps, list(range(N_CORES)))
    out = np.stack(
        [np.asarray(res.results[b]["out"]).astype(np.float32) for b in range(N_CORES)]
    )
    avg = np.stack(
        [np.asarray(res.results[b]["avg"]).astype(np.float32) for b in range(N_CORES)]
    )
    return out, avg
